# revision 2
# baseline (speedup 1.0000x reference)
"""CTC batch loss (Keras convention, blank = C-1) on 8 Trainium2 NeuronCores.

v4 (pure data parallel, 128 examples per core = 128 SBUF partitions):
  * Stage 1: SWDGE cast f32->bf16 DRAM->DRAM; 16 xbar transposes (8 examples
    each) -> xt [c, (e t)]; HWDGE stores -> ytr[g, c, e, t] DRAM.
  * p_store [b-part, s, t] via 4 dma_gathers of 512B rows (b, ext[b,s]),
    host-computed int16 indices. Their Q7 descriptor-generation (~8ns/idx,
    66us total) is PREPARED on the idle Pool engine during stage-1 transfers
    and TRIGGERED after the stores land (dsem + all-engine barrier).
  * DP in prob domain, fwd/bwd meet-in-middle (127 rounds). State split into
    STA (odd states, guard+132) / STE (even states, 132) tiles + a shifted
    copy A (AOs) maintained on ACT, so every DVE op is an aligned even bf16
    TT in 2x mode reading distinct tiles, ordered to avoid pipe-drain
    stalls. Per-round strips PPP[rho] = [POC|PB|PM] prebuilt on ACT/DVE with
    the reference's +1e-7 eps folded in via activation bias.
  * Rescale every 8 rounds; log corrections applied once at the end.
"""

import sys
from contextlib import ExitStack

import numpy as np

for _p in ("/opt/trn_rl_repo",):
    if _p not in sys.path:
        sys.path.insert(0, _p)

import concourse.bass as bass
import concourse.tile as tile
from concourse import mybir
from concourse.bass import broadcast_tensor_aps
from concourse.bass_utils import run_bass_kernel_spmd

B, T, C, L = 1024, 256, 128, 64
NCORES = 8
BL = B // NCORES
S = 65
EPS = 1e-7
NR = 128
W = 132
SW = 3 * W

f32 = mybir.dt.float32
bf16 = mybir.dt.bfloat16
i16 = mybir.dt.int16

ADD = mybir.AluOpType.add
MULT = mybir.AluOpType.mult
AX_X = mybir.AxisListType.X
AFT = mybir.ActivationFunctionType
Copy = AFT.Copy
PRE = 128.0               # deterministic per-round prescale (2^7)
NSNAP = 9


def _body(tc, loss_ap, yp, ybf, idx2_ap, mcat_ap, ytr):
    nc = tc.nc
    with ExitStack() as ctx:
        const = ctx.enter_context(tc.tile_pool(name="const", bufs=1))
        xtp = ctx.enter_context(tc.tile_pool(name="xt", bufs=3))
        tiny = ctx.enter_context(tc.tile_pool(name="tiny", bufs=1))

        PPP = const.tile([128, (NR - 1) * SW], bf16)
        p_store = const.tile([128, S * T], bf16)
        mcat = const.tile([128, W], bf16)
        idx2 = const.tile([128, 520], i16)
        STA = [const.tile([128, 134], bf16, name=f"sta{i}") for i in range(2)]
        STE = [const.tile([128, 132], bf16, name=f"ste{i}") for i in range(2)]
        AOs = [const.tile([128, 134], bf16, name=f"aos{i}") for i in range(2)]
        csstore = const.tile([128, NSNAP], f32)

        nc.sync.dma_start(idx2[:], idx2_ap[:, :])
        nc.sync.dma_start(mcat[:], mcat_ap[:, :])

        dsem = nc.alloc_semaphore(name="g2dma")
        nc.gpsimd.sem_clear(dsem)
        tc.no_sync_barrier()

        # ---- stage 1a: cast DMAs (SWDGE, Pool DGE first) ----
        for k in range(8):
            b0, b1 = k * 16, (k + 1) * 16
            nc.gpsimd.dma_start(ybf[b0:b1, :, :], yp[b0:b1, :, :])

        # ---- gather2 DGE prepared early on the Pool engine ----
        ps3 = p_store[:].rearrange("p (s t) -> p s t", s=S)
        ytr_rows = ytr.rearrange("g c e t -> (g c e) t")
        SBLK = [(0, 16), (16, 32), (32, 48), (48, 65)]
        for s0, s1 in SBLK:
            n = (s1 - s0) * 128
            nc.gpsimd.dma_gather(
                ps3[:, s0:s1, :],
                ytr_rows,
                idx2[:, s0 * 8 : s0 * 8 + n // 16],
                num_idxs=n,
                num_idxs_reg=n,
                elem_size=T,
                single_packet=False,
                prepare_only=True,
                sem=dsem,
            )

        # ---- stage 1b: transposes + stores (HWDGE queues) ----
        for g in range(16):
            xt = xtp.tile([128, 8 * T], bf16, name="xt")
            eng = nc.sync if g % 2 == 0 else nc.scalar
            oeng = nc.scalar if g % 2 == 0 else nc.sync
            eng.dma_start_transpose(
                xt[:], ybf[g * 8 : (g + 1) * 8, :, :].rearrange("b t c -> (b t) c")
            )
            oeng.dma_start(ytr[g, :, :, :], xt[:].rearrange("c (e t) -> c e t", e=8))

        # ---- trigger gather2 after stores land ----
        dumt = tiny.tile([16, T], bf16)
        dumt2 = tiny.tile([16, T], bf16)
        nc.sync.dma_start(dumt[:], ytr[:, 0, 0, :])
        nc.gpsimd.tensor_copy(dumt2[:], dumt[:])
        tc.no_sync_barrier()
        nc.gpsimd.trigger_dma(count=4)
        nc.gpsimd.wait_ge(dsem, 64)
        tc.strict_bb_all_engine_barrier()

        # ---- strips ----
        PPP3 = PPP[:].rearrange("p (r c) -> p r c", c=SW)
        ps_ts = p_store[:].rearrange("p (s t) -> p t s", s=S)
        nc.vector.memset(PPP3[:, :, 64:66], 0.0)
        nc.vector.memset(PPP3[:, :, 130:132], 0.0)

        RB = [(0, 32), (32, 64), (64, 96), (96, 127)]

        def build_poc_pb(r):
            R0, R1 = RB[r]
            o3 = PPP3[:, R0:R1, :]
            nc.scalar.activation(
                o3[:, :, 0:64], ps_ts[:, R0 + 1 : R1 + 1, 0:64], Copy,
                bias=PRE * EPS, scale=PRE,
            )
            nc.scalar.activation(
                o3[:, :, 66:130],
                ps_ts[:, 254 - R0 : 254 - R1 : -1, 63::-1],
                Copy,
                bias=PRE * EPS, scale=PRE,
            )
            a, bb = broadcast_tensor_aps(
                o3[:, :, 132:198], ps_ts[:, R0 + 1 : R1 + 1, 64:65]
            )
            nc.scalar.activation(a, bb, Copy, bias=PRE * EPS, scale=PRE)
            a, bb = broadcast_tensor_aps(
                o3[:, :, 198:264], ps_ts[:, 254 - R0 : 254 - R1 : -1, 64:65]
            )
            nc.scalar.activation(a, bb, Copy, bias=PRE * EPS, scale=PRE)

        def build_pm(r):
            R0, R1 = RB[r]
            i0, i1 = broadcast_tensor_aps(
                PPP3[:, R0:R1, 0:132], mcat[:].rearrange("p (r c) -> p r c", r=1)
            )
            nc.gpsimd.tensor_tensor(PPP3[:, R0:R1, 264:396], i0, i1, MULT)

        build_poc_pb(0)
        build_pm(0)

        # ---- init ----
        for i in range(2):
            nc.vector.memset(STA[i][:], 0.0)
            nc.vector.memset(STE[i][:], 0.0)
            nc.vector.memset(AOs[i][:], 0.0)
        nc.scalar.activation(STE[0][:, 0:1], ps3[:, 64:65, 0:1], Copy, bias=PRE * EPS, scale=PRE)
        nc.scalar.activation(STA[0][:, 2:3], ps3[:, 0:1, 0:1], Copy, bias=PRE * EPS, scale=PRE)
        nc.scalar.activation(STE[0][:, 66:67], ps3[:, 64:65, 255:256], Copy, bias=PRE * EPS, scale=PRE)
        nc.scalar.activation(STA[0][:, 68:69], ps3[:, 63:64, 255:256], Copy, bias=PRE * EPS, scale=PRE)
        # AOs for round 1 = shift of round-0 AO
        nc.vector.tensor_copy(AOs[1][:, 1:133], STA[0][:, 2:134])

        # ---- DP ----
        U1 = tiny.tile([128, W], bf16)
        U2s = [tiny.tile([128, W], bf16, name=f"u2_{i}") for i in range(2)]
        Z = tiny.tile([128, W], bf16)
        Q = tiny.tile([128, W], bf16)
        cs = tiny.tile([128, 1], f32)
        cs2 = tiny.tile([128, 1], f32)
        rrec = tiny.tile([128, 1], f32)

        SNAPS = sorted(set(range(16, 113, 16)) | {120, 127})
        TRIGGER = {8: 1, 40: 2, 72: 3}
        k_resc = 0
        for rho in range(1, NR):
            PA, NA = STA[(rho - 1) % 2], STA[rho % 2]
            PE, NE = STE[(rho - 1) % 2], STE[rho % 2]
            A = AOs[rho % 2]          # holds shift of AO(rho-1)
            An = AOs[(rho + 1) % 2]   # to be filled with shift of AO(rho)
            U2 = U2s[rho % 2]
            base = (rho - 1) * SW
            # all-DVE round, every op reads >=2 ops back (no pipe-drain stalls)
            nc.vector.tensor_tensor(U1[:], PA[:, 2:134], PE[:], ADD)
            nc.vector.tensor_tensor(Q[:], A[:, 0:132], PPP[:, base + 2 * W : base + 3 * W], MULT)
            nc.vector.tensor_tensor(Z[:], U1[:], PPP[:, base : base + W], MULT)
            nc.vector.tensor_tensor(U2[:], PE[:], A[:, 0:132], ADD)
            nc.vector.tensor_tensor(NA[:, 2:134], Z[:], Q[:], ADD)
            nc.vector.tensor_tensor(NE[:], U2[:], PPP[:, base + W : base + 2 * W], MULT)
            if rho in TRIGGER:
                build_poc_pb(TRIGGER[rho])
                build_pm(TRIGGER[rho])
            if rho in SNAPS:
                nc.vector.tensor_reduce(cs[:], NA[:, 2:134], AX_X, ADD)
                nc.vector.tensor_reduce(cs2[:], NE[:], AX_X, ADD)
                nc.vector.tensor_tensor(cs[:], cs[:], cs2[:], ADD)
                nc.vector.tensor_copy(csstore[:, k_resc : k_resc + 1], cs[:])
                nc.vector.reciprocal(rrec[:], cs[:])
                nc.vector.tensor_scalar(NA[:, 2:134], NA[:, 2:134], rrec[:], None, MULT)
                nc.vector.tensor_scalar(NE[:], NE[:], rrec[:], None, MULT)
                k_resc += 1
            # shifted AO copy for the next round (reads final, possibly rescaled NA)
            nc.vector.tensor_copy(An[:, 1:133], NA[:, 2:134])
        assert k_resc == NSNAP

        # ---- endgame ----
        FA = STA[(NR - 1) % 2]
        FE = STE[(NR - 1) % 2]
        UF = tiny.tile([128, 64], bf16)
        QF = tiny.tile([128, 64], bf16)
        VF = tiny.tile([128, 64], bf16)
        UE = tiny.tile([128, 65], bf16)
        D = tiny.tile([128, 130], bf16)
        nc.vector.tensor_tensor(UF[:], FA[:, 2:66], FE[:, 0:64], ADD)
        nc.vector.tensor_tensor(QF[:], FA[:, 1:65], mcat[:, 0:64], MULT)
        nc.vector.tensor_tensor(VF[:], UF[:], QF[:], ADD)
        nc.vector.tensor_tensor(UE[:], FE[:, 0:65], FA[:, 1:66], ADD)
        nc.vector.tensor_tensor(D[:, 0:64], VF[:], FA[:, 131:67:-1], MULT)
        nc.vector.tensor_tensor(D[:, 64:129], UE[:], FE[:, 130:65:-1], MULT)
        lik = tiny.tile([128, 1], f32)
        nc.vector.tensor_reduce(lik[:], D[:, 0:129], AX_X, ADD)
        lik2 = tiny.tile([128, 1], f32)
        nc.vector.tensor_scalar(lik2[:], lik[:], float(2.0 ** 80), None, MULT)
        lnlik = tiny.tile([128, 1], f32)
        nc.scalar.activation(lnlik[:], lik2[:], AFT.Ln)
        strip = tiny.tile([128, NSNAP], f32)
        nc.scalar.activation(strip[:], csstore[:], AFT.Ln)
        ssum = tiny.tile([128, 1], f32)
        nc.vector.tensor_reduce(ssum[:], strip[:], AX_X, ADD)
        # 256 p-slices each carried a x128 prescale: ln lik_true = ln lik_comp
        # + 2*sum(ln cs) - 256*ln(128)
        CADD = float(256 * np.log(PRE) + 80 * np.log(2.0))
        t1 = tiny.tile([128, 1], f32)
        nc.vector.tensor_scalar(t1[:], ssum[:], -2.0, CADD, MULT, ADD)
        lout = tiny.tile([128, 1], f32)
        nc.vector.scalar_tensor_tensor(lout[:], lnlik[:], -1.0, t1[:], MULT, ADD)
        nc.sync.dma_start(loss_ap[:, :], lout[:])


def finalize_libraries(nc):
    from concourse.library_config import all_libraries, standard
    from concourse.bass import _bass_rust

    mask = {}
    for lib in all_libraries:
        for it in lib.instructions:
            mask[it] = mask.get(it, 0) | (1 << lib.index)
    _bass_rust.insert_library_loads(nc, mask, len(all_libraries), standard.index)
    mybir.codegen_inst_isa_subclasses(nc)


def build_nc():
    nc = bass.Bass("TRN2", target_bir_lowering=False, debug=False)
    yp = nc.dram_tensor("y_pred", [BL, T, C], f32, kind="ExternalInput").ap()
    idx2_in = nc.dram_tensor("idx2", [128, 520], i16, kind="ExternalInput").ap()
    mc_in = nc.dram_tensor("m_cat", [128, W], bf16, kind="ExternalInput").ap()
    ybf = nc.dram_tensor("ybf", [BL, T, C], bf16, kind="Internal").ap()
    ytr = nc.dram_tensor("ytr", [16, C, 8, T], bf16, kind="Internal").ap()
    loss = nc.dram_tensor("loss", [BL, 1], f32, kind="ExternalOutput").ap()
    with tile.TileContext(nc) as tc:
        _body(tc, loss, yp, ybf, idx2_in, mc_in, ytr)
    finalize_libraries(nc)
    return nc


def host_consts(y_true):
    """Per-core gather indices and skip-mask tiles (label functions only)."""
    import ml_dtypes

    lab = np.asarray(y_true).astype(np.int64)
    iarr = np.arange(S * 128)
    outs = []
    for ci in range(NCORES):
        lb = lab[ci * BL : (ci + 1) * BL]
        ext = np.concatenate([lb, np.full((BL, 1), C - 1, np.int64)], axis=1)
        b = np.arange(BL)
        # row = g*1024 + c*8 + e ; b = g*8 + e
        row = (b // 8)[:, None] * 1024 + ext * 8 + (b % 8)[:, None]  # [b, s]
        flat = row.T.reshape(-1)  # i = s*128 + b
        assert flat.max() < 2 ** 15
        w2 = np.zeros((16, 520), np.int16)
        w2[iarr % 16, iarr // 16] = flat.astype(np.int16)
        idx2_t = np.tile(w2, (8, 1))

        mc = np.zeros((BL, W), np.float32)
        dif = (lb[:, 1:] != lb[:, :-1]).astype(np.float32)
        mc[:, 1:64] = dif
        mc[:, 67:130] = dif[:, ::-1]
        outs.append((idx2_t, mc.astype(ml_dtypes.bfloat16)))
    return outs


_CACHE = {}

_SPLIT_OPS = {"DMACopy", "DmaTransposeAnt", "DMAGatherAnt", "Drain", "NoOp"}


def _legalize_bir(bir_bytes):
    import orjson

    d = orjson.loads(bir_bytes)
    n_new = 0
    for fn in d.get("functions", []):
        for blk in fn.get("blocks", []):
            insts = blk.get("instructions")
            if not insts:
                continue
            out = []
            for ins in insts:
                si = ins.get("sync_info")
                if si:
                    waits = si.get("on_wait") or []
                    if len(waits) > 1:
                        for w in waits[:-1]:
                            n_new += 1
                            out.append(
                                {
                                    "debug": ins.get("debug", 0),
                                    "engine": ins["engine"],
                                    "ins": [],
                                    "outs": [],
                                    "name": f"ZW-{n_new}",
                                    "opcode": "NoOp",
                                    "sync_info": {"on_wait": [w], "on_update": []},
                                }
                            )
                        si["on_wait"] = [waits[-1]]
                out.append(ins)
            blk["instructions"] = out
    return orjson.dumps(d)


def _install_bir_legalizer():
    import concourse.bass2jax as b2j

    if getattr(b2j, "_ctc_legalizer_installed", False):
        return
    orig = b2j.compile_bir_kernel

    def wrapper(bir_json, tmpdir, neff_name="file.neff"):
        bir_json = _legalize_bir(bir_json)
        return orig(bir_json, tmpdir, neff_name=neff_name)

    b2j.compile_bir_kernel = wrapper
    b2j._ctc_legalizer_installed = True


def kernel(y_true, y_pred):
    assert y_pred.shape == (B, T, C) and y_true.shape == (B, L)
    _install_bir_legalizer()
    nc = _CACHE.get("nc")
    if nc is None:
        nc = _CACHE["nc"] = build_nc()
    yp = np.ascontiguousarray(y_pred, dtype=np.float32)
    consts = host_consts(y_true)
    in_maps = [
        {
            "y_pred": yp[i * BL : (i + 1) * BL],
            "idx2": consts[i][0],
            "m_cat": consts[i][1],
        }
        for i in range(NCORES)
    ]
    res = run_bass_kernel_spmd(nc, in_maps, list(range(NCORES)))
    out = np.concatenate([res.results[i]["loss"] for i in range(NCORES)], axis=0)
    return out.astype(np.float32)
